# revision 1
# baseline (speedup 1.0000x reference)
"""Trainium2 Bass kernel for multi-head attention with adaptive span masking.

Computation (per the nn.Module):
    q = (query @ Wq.T) split into B*H rows of size d=64
    attn = softmax((key . q + q @ key_pe) / sqrt(d))
    attn = renormalize(attn * adaptive_span_mask)
    out = (attn . value) merged heads @ Wo.T

Sharding: batch-parallel across 8 cores. Core c gets batches [4c, 4c+4)
(all 8 heads) = rows [32c, 32c+32) of key/value; Wq/Wo/key_pe/span are
replicated. Each core produces its own [4, 512] output block; the host
concatenates. No collectives needed.
"""

import math
import os
import sys

import numpy as np

for _p in ("/opt/trn_rl_repo", "/root/.axon_site/_ro/trn_rl_repo"):
    if os.path.isdir(_p) and _p not in sys.path:
        sys.path.insert(0, _p)

import concourse.bass as bass
import concourse.bacc as bacc
import concourse.mybir as mybir
from concourse.bass import ts
from concourse.masks import make_identity
from concourse.tile import TileContext

F32 = mybir.dt.float32

# Problem constants (hardcoded per contest contract)
NHEADS = 8
HEAD_DIM = 64
HID = NHEADS * HEAD_DIM  # 512
B = 32
M = 8192
RAMP = 32.0

N_CORES = 8
BPC = B // N_CORES        # 4 batches per core
NPC = BPC * NHEADS        # 32 (b,h) rows per core
MO = M // 128             # 64 m-blocks of 64 within each partition's range
ROWS_PER_DMA = 2          # kv rows fetched per dma_start (4 MB transfers)

_CACHE = {}


def build_nc():
    nc = bacc.Bacc(None, target_bir_lowering=False)
    AF = mybir.ActivationFunctionType
    ALU = mybir.AluOpType
    BF16 = mybir.dt.bfloat16

    q_d = nc.dram_tensor("query", [BPC, HID], F32, kind="ExternalInput")
    k_d = nc.dram_tensor("key", [NPC, M, HEAD_DIM], F32, kind="ExternalInput")
    v_d = nc.dram_tensor("value", [NPC, M, HEAD_DIM], F32, kind="ExternalInput")
    wq_d = nc.dram_tensor("Wq", [HID, HID], F32, kind="ExternalInput")
    wo_d = nc.dram_tensor("Wo", [HID, HID], F32, kind="ExternalInput")
    kpe_d = nc.dram_tensor("key_pe", [HEAD_DIM, M], F32, kind="ExternalInput")
    span_d = nc.dram_tensor("span", [NHEADS, 1], F32, kind="ExternalInput")
    out_d = nc.dram_tensor("out", [BPC, HID], F32, kind="ExternalOutput")

    with TileContext(nc) as tc:
        with (
            tc.tile_pool(name="persist", bufs=1) as persist,
            # main-loop pools created BEFORE setup pools so the kv DMAs get
            # SBUF ranges disjoint from setup tiles (no WAR dep -> kv loads
            # start at t=0, overlapping the whole setup phase)
            tc.tile_pool(name="kv", bufs=2) as kv_pool,
            tc.tile_pool(name="sc", bufs=3) as sc_pool,
            tc.tile_pool(name="fin", bufs=1) as fin_pool,
            tc.tile_pool(name="ps_s", bufs=2, space="PSUM") as ps_s_pool,
            tc.tile_pool(name="ps_o", bufs=2, space="PSUM") as ps_o_pool,
            tc.tile_pool(name="ps_fin", bufs=1, space="PSUM") as ps_fin_pool,
        ):
            identity = persist.tile([128, 128], F32, tag="identity")
            make_identity(nc, identity[:])
            ones_row = persist.tile([1, 128], F32, tag="ones_row")
            nc.vector.memset(ones_row[:], 1.0)
            ones_col = persist.tile([128, 1], F32, tag="ones_col")
            nc.vector.memset(ones_col[:], 1.0)

            woT = [persist.tile([128, HID], F32, name=f"woT{j}", tag=f"woT{j}") for j in range(4)]
            q_sb = persist.tile([BPC, HID], F32, tag="q_sb")
            qts = persist.tile([HEAD_DIM, BPC, NHEADS], F32, tag="qts")
            qrep = persist.tile([128, BPC, HID], F32, tag="qrep")
            pos_all = persist.tile([128, MO, NPC], F32, tag="pos_all")
            masks = persist.tile([128, NHEADS, MO], F32, tag="masks")
            ao_sb = persist.tile([1, BPC, HID], F32, tag="ao_sb")

            # ---------------- setup phase A: weight transposes + q ----------
            with (
                tc.tile_pool(name="setupA", bufs=1) as sa,
                tc.tile_pool(name="psA", bufs=2, space="PSUM") as psA,
            ):
                wqT = [sa.tile([128, HID], F32, name=f"wqT{j}", tag=f"wqT{j}") for j in range(4)]
                wq_sb = [sa.tile([128, HID], F32, name=f"wq_sb{i}", tag="wq_sb", bufs=2) for i in range(4)]
                wo_sb = [sa.tile([128, HID], F32, name=f"wo_sb{i}", tag="wo_sb", bufs=2) for i in range(4)]
                for i in range(4):
                    nc.sync.dma_start(out=wq_sb[i][:], in_=wq_d[ts(i, 128), :])
                    nc.sync.dma_start(out=wo_sb[i][:], in_=wo_d[ts(i, 128), :])
                for io in range(4):
                    for jo in range(4):
                        pwt = psA.tile([128, 128], F32, tag="pwt")
                        nc.tensor.matmul(
                            pwt[:], wq_sb[io][:, ts(jo, 128)], identity[:],
                            start=True, stop=True,
                        )
                        nc.scalar.copy(wqT[jo][:, ts(io, 128)], pwt[:])
                        pwt2 = psA.tile([128, 128], F32, tag="pwt")
                        nc.tensor.matmul(
                            pwt2[:], wo_sb[io][:, ts(jo, 128)], identity[:],
                            start=True, stop=True,
                        )
                        nc.scalar.copy(woT[jo][:, ts(io, 128)], pwt2[:])

                query_sb = sa.tile([BPC, HID], F32, tag="query_sb")
                nc.sync.dma_start(out=query_sb[:], in_=q_d[:])
                qTq = [sa.tile([128, BPC], F32, name=f"qTq{j}", tag=f"qTq{j}") for j in range(4)]
                for jo in range(4):
                    pqt = psA.tile([128, BPC], F32, tag="pwt")
                    nc.tensor.matmul(
                        pqt[:], query_sb[:, ts(jo, 128)], identity[0:BPC, 0:BPC],
                        start=True, stop=True,
                    )
                    nc.scalar.copy(qTq[jo][:], pqt[:])
                # q = query @ Wq.T  ->  [4, 512]
                ps_q = psA.tile([BPC, HID], F32, tag="ps_q", bufs=1)
                for jo in range(4):
                    nc.tensor.matmul(
                        ps_q[:], qTq[jo][:], wqT[jo][:],
                        start=(jo == 0), stop=(jo == 3),
                    )
                nc.scalar.copy(q_sb[:], ps_q[:])
                # qts[d, b, h] = q[b, h*64+d]   (64 partitions)
                for h in range(NHEADS):
                    pqh = psA.tile([HEAD_DIM, BPC], F32, tag="pwt")
                    nc.tensor.matmul(
                        pqh[:], q_sb[:, ts(h, HEAD_DIM)], identity[0:BPC, 0:BPC],
                        start=True, stop=True,
                    )
                    nc.scalar.copy(qts[:, :, h], pqh[:])

            # ---------------- setup phase B: qrep, pos, masks ---------------
            with (
                tc.tile_pool(name="setupB", bufs=1) as sb,
                tc.tile_pool(name="psB", bufs=2, space="PSUM") as psB,
            ):
                # q replicated across partitions: qrep[p, b, :] = q[b, :]
                # (bounce via DRAM -- DMA partition-broadcast needs a DRAM src)
                with tc.tile_pool(name="dramq", bufs=1, space="DRAM") as dq:
                    q_dram = dq.tile([BPC, HID], F32, tag="q_dram")
                    nc.sync.dma_start(out=q_dram[:], in_=q_sb[:])
                    for b in range(BPC):
                        nc.gpsimd.dma_start(
                            out=qrep[:, b, :],
                            in_=q_dram[b : b + 1, :].partition_broadcast(128),
                        )

                # positional scores: pos[p, mo, n] = sum_d key_pe[d, 64p+mo] * q[n, d]
                kpe_sb = sb.tile([HEAD_DIM, M], F32, tag="kpe_sb")
                nc.sync.dma_start(out=kpe_sb[:], in_=kpe_d[:])
                kpe_r = kpe_sb[:].rearrange("d (p mo) -> d mo p", mo=MO)
                for mog in range(MO // 16):
                    ps_p = psB.tile([128, 16, NPC], F32, tag="ps_p")
                    for k in range(16):
                        mo = mog * 16 + k
                        nc.tensor.matmul(
                            ps_p[:, k, :], kpe_r[:, mo, :], qts[:],
                            start=True, stop=True,
                        )
                    nc.scalar.copy(pos_all[:, ts(mog, 16), :], ps_p[:])

                # masks[p, h, mo] = clip((64p+mo)/32 + span[h]*256 - 254.96875, 0, 1)
                m_f = sb.tile([128, MO], F32, tag="m_f")
                nc.gpsimd.iota(
                    out=m_f[:], pattern=[[1, MO]], base=0,
                    channel_multiplier=MO,
                    allow_small_or_imprecise_dtypes=True,
                )
                span_row = sb.tile([1, NHEADS], F32, tag="span_row")
                nc.sync.dma_start(out=span_row[:], in_=span_d[:].rearrange("h o -> o h"))
                ps_sp = psB.tile([128, NHEADS], F32, tag="ps_sp", bufs=1)
                nc.tensor.matmul(
                    ps_sp[:], ones_row[:], span_row[:], start=True, stop=True
                )
                span_b = sb.tile([128, NHEADS], F32, tag="span_b")
                bias_const = float(-(M - 1) / RAMP + 1.0)  # -254.96875
                nc.scalar.activation(
                    out=span_b[:], in_=ps_sp[:], func=AF.Copy,
                    scale=float(M / RAMP), bias=bias_const,
                )
                for h in range(NHEADS):
                    nc.scalar.activation(
                        out=masks[:, h, :], in_=m_f[:], func=AF.Identity,
                        scale=float(1.0 / RAMP), bias=span_b[:, h : h + 1],
                    )
                    nc.vector.tensor_scalar(
                        out=masks[:, h, :], in0=masks[:, h, :],
                        scalar1=0.0, scalar2=1.0,
                        op0=ALU.max, op1=ALU.min,
                    )

            # ---------------- main loop over (b, h) rows --------------------
            for i in range(NPC):
                b, h = divmod(i, NHEADS)
                kt = kv_pool.tile([128, MO, HEAD_DIM], F32, tag="kt")
                vt = kv_pool.tile([128, MO, HEAD_DIM], F32, tag="vt")
                nc.sync.dma_start(
                    out=kt[:],
                    in_=k_d[i].rearrange("(p mo) d -> p mo d", p=128),
                )
                nc.scalar.dma_start(
                    out=vt[:],
                    in_=v_d[i].rearrange("(p mo) d -> p mo d", p=128),
                )
                # f32 -> bf16 cast on the scalar engine (ACT has slack);
                # bf16 halves PE work in the PV matmuls
                vtb = kv_pool.tile([128, MO, HEAD_DIM], BF16, tag="vtb")
                nc.scalar.copy(vtb[:], vt[:])
                # content + positional scores:
                # scores[p, mo] = pos[p, mo, i] + sum_d key[..] * q[i, d]
                prod = sc_pool.tile([128, MO, HEAD_DIM], F32, tag="prod", bufs=1)
                q_b = (
                    qrep[:, b, ts(h, HEAD_DIM)]
                    .rearrange("p (x d) -> p x d", x=1)
                    .broadcast_to((128, MO, HEAD_DIM))
                )
                nc.vector.tensor_mul(prod[:], kt[:], q_b)
                scores = sc_pool.tile([128, MO], F32, tag="scores")
                nc.vector.reduce_sum(scores[:], prod[:], axis=mybir.AxisListType.X)
                nc.vector.tensor_add(scores[:], scores[:], pos_all[:, :, i])
                # e = exp(scores / sqrt(d)), Sigma_e fused
                e_t = sc_pool.tile([128, MO], F32, tag="e_t")
                sums = sc_pool.tile([128, 2], F32, tag="sums")
                nc.scalar.activation(
                    out=e_t[:], in_=scores[:], func=AF.Exp,
                    scale=float(1.0 / math.sqrt(HEAD_DIM)),
                    accum_out=sums[:, 0:1],
                )
                # w = e * mask[h] (bf16 for the PE), then Sigma_w
                w_t = sc_pool.tile([128, MO], BF16, tag="w_t")
                nc.vector.tensor_mul(w_t[:], e_t[:], masks[:, h, :])
                nc.vector.reduce_sum(
                    sums[:, 1:2], w_t[:], axis=mybir.AxisListType.X
                )
                # partition-reduce both sums: [1, 2] = ones.T @ sums
                ps_s = ps_s_pool.tile([1, 2], F32, tag="ps_s")
                nc.tensor.matmul(
                    ps_s[:], ones_col[:], sums[:], start=True, stop=True
                )
                sums_sb = sc_pool.tile([1, 2], F32, tag="sums_sb")
                nc.scalar.copy(sums_sb[:], ps_s[:])
                # u = Sigma_w + 1e-8 * Sigma_e ; scal = 1/u
                u_t = sc_pool.tile([1, 1], F32, tag="u_t")
                nc.scalar.activation(
                    out=u_t[:], in_=sums_sb[:, 0:1], func=AF.Identity,
                    scale=1e-8, bias=sums_sb[:, 1:2],
                )
                scal = sc_pool.tile([1, 1], F32, tag="scal")
                nc.vector.reciprocal(scal[:], u_t[:])
                # out_row = sum_m w[m] * value[m, :]   (bf16 PE, PSUM accum)
                ps_o = ps_o_pool.tile([1, HEAD_DIM], F32, tag="ps_o")
                for mo in range(MO):
                    nc.tensor.matmul(
                        ps_o[:],
                        w_t[:, mo : mo + 1],
                        vtb[:, mo, :],
                        start=(mo == 0),
                        stop=(mo == MO - 1),
                    )
                # ao[0, b, h*64:(h+1)*64] = ps_o * scal
                nc.scalar.activation(
                    out=ao_sb[0:1, b, ts(h, HEAD_DIM)], in_=ps_o[:],
                    func=AF.Copy, scale=scal[:, 0:1],
                )

            # ---------------- output projection -------------------------
            aoT = []
            for co in range(4):
                ps_t2 = ps_fin_pool.tile([128, BPC], F32, name="ps_t2", tag="ps_fin")
                for b in range(BPC):
                    nc.tensor.matmul(
                        ps_t2[:, b : b + 1],
                        ao_sb[0:1, b, ts(co, 128)],
                        identity[0:1, 0:1],
                        start=True, stop=True,
                    )
                t_sb = fin_pool.tile([128, BPC], F32, name=f"t_sb{co}", tag=f"t_sb{co}")
                nc.scalar.copy(t_sb[:], ps_t2[:])
                aoT.append(t_sb)
            ps_f = ps_fin_pool.tile([BPC, HID], F32, name="ps_f", tag="ps_fin")
            for co in range(4):
                nc.tensor.matmul(
                    ps_f[:], aoT[co][:], woT[co][:],
                    start=(co == 0), stop=(co == 3),
                )
            out_sb = fin_pool.tile([BPC, HID], F32, tag="out_sb")
            nc.scalar.copy(out_sb[:], ps_f[:])
            nc.sync.dma_start(out=out_d[:], in_=out_sb[:])

    nc.compile()
    return nc


def _get_nc():
    if "nc" not in _CACHE:
        _CACHE["nc"] = build_nc()
    return _CACHE["nc"]


def _make_in_maps(query, key, value, Wq, Wo, key_pe, span):
    q2 = np.ascontiguousarray(np.asarray(query, np.float32).reshape(B, HID))
    key = np.asarray(key, np.float32)
    value = np.asarray(value, np.float32)
    Wq = np.ascontiguousarray(np.asarray(Wq, np.float32))
    Wo = np.ascontiguousarray(np.asarray(Wo, np.float32))
    key_pe = np.ascontiguousarray(np.asarray(key_pe, np.float32))
    span = np.ascontiguousarray(np.asarray(span, np.float32))
    in_maps = []
    for c in range(N_CORES):
        in_maps.append(
            {
                "query": np.ascontiguousarray(q2[c * BPC : (c + 1) * BPC]),
                "key": np.ascontiguousarray(key[c * NPC : (c + 1) * NPC]),
                "value": np.ascontiguousarray(value[c * NPC : (c + 1) * NPC]),
                "Wq": Wq,
                "Wo": Wo,
                "key_pe": key_pe,
                "span": span,
            }
        )
    return in_maps


def _install_ntff_hook():
    """Shim antenv.axon_hooks with a ctypes NTFF profile hook so
    run_bass_kernel_spmd(trace=True) works in this container."""
    import contextlib
    import ctypes
    import types

    try:
        import antenv.axon_hooks  # noqa: F401

        return
    except ImportError:
        pass
    so_path = "/opt/axon/libaxon_pjrt.so"
    import antenv

    mod = types.ModuleType("antenv.axon_hooks")
    holder = {"hook": None}

    if os.path.exists(so_path):
        lib = ctypes.CDLL(so_path)
        if hasattr(lib, "axon_start_nrt_profile"):
            lib.axon_start_nrt_profile.argtypes = [
                ctypes.POINTER(ctypes.c_int64),
                ctypes.c_size_t,
            ]
            lib.axon_start_nrt_profile.restype = ctypes.c_int64
            lib.axon_stop_nrt_profile.argtypes = [ctypes.c_char_p]
            lib.axon_stop_nrt_profile.restype = ctypes.c_int64

            @contextlib.contextmanager
            def _hook(output_dir, device_ids):
                import jax

                jax.devices()
                if device_ids:
                    ids = (ctypes.c_int64 * len(device_ids))(*device_ids)
                    rc = lib.axon_start_nrt_profile(ids, len(device_ids))
                else:
                    rc = lib.axon_start_nrt_profile(None, 0)
                if rc != 0:
                    raise RuntimeError(f"axon_start_nrt_profile rc={rc}")
                try:
                    yield
                finally:
                    n = lib.axon_stop_nrt_profile(str(output_dir).encode())
                    print(f"profile: {n} file(s) written to {output_dir}")

            holder["hook"] = _hook

    mod.get_axon_ntff_profile_hook = lambda: holder["hook"]
    mod.set_axon_ntff_profile_hook = lambda h: holder.__setitem__("hook", h)
    sys.modules["antenv.axon_hooks"] = mod
    antenv.axon_hooks = mod


def run(query, key, value, Wq, Wo, key_pe, span, trace=False):
    """Run on hardware; returns (output [B,1,HID], BassKernelResults)."""
    from concourse import bass_utils
    from concourse.bass_utils import run_bass_kernel_spmd

    if trace:
        _install_ntff_hook()
        bass_utils.upload_artifacts = lambda tmpdir: f"local:{tmpdir}"
    nc = _get_nc()
    in_maps = _make_in_maps(query, key, value, Wq, Wo, key_pe, span)
    res = run_bass_kernel_spmd(nc, in_maps, list(range(N_CORES)), trace=trace)
    out = np.concatenate(
        [np.asarray(res.results[c]["out"]) for c in range(N_CORES)], axis=0
    )
    return out.reshape(B, 1, HID).astype(np.float32), res


def kernel(query, key, value, Wq, Wo, key_pe, span):
    out, _ = run(query, key, value, Wq, Wo, key_pe, span, trace=False)
    return out


def run_timed(query, key, value, Wq, Wo, key_pe, span, iters=6):
    """Run via a cached sharded PJRT executable with device-staged inputs.

    Returns (output [B,1,HID], list of per-iteration wall times in seconds).
    The first iteration (compile+transfer) is excluded from the returned
    times; inputs stay on device so later iterations time dispatch + device
    execution only.
    """
    import time as _time

    import jax
    from jax.sharding import Mesh, NamedSharding, PartitionSpec
    from jax.experimental.shard_map import shard_map
    from concourse import bass2jax
    from concourse import mybir as _mb

    nc = _get_nc()
    bass2jax.install_neuronx_cc_hook()
    in_maps = _make_in_maps(query, key, value, Wq, Wo, key_pe, span)
    n_cores = N_CORES

    partition_name = (
        nc.partition_id_tensor.name if nc.partition_id_tensor else None
    )
    in_names, out_names, out_avals, zero_outs = [], [], [], []
    for alloc in nc.m.functions[0].allocations:
        if not isinstance(alloc, _mb.MemoryLocationSet):
            continue
        name = alloc.memorylocations[0].name
        if alloc.kind == "ExternalInput":
            if name != partition_name:
                in_names.append(name)
        elif alloc.kind == "ExternalOutput":
            shape = tuple(alloc.tensor_shape)
            dtype = _mb.dt.np(alloc.dtype)
            out_names.append(name)
            out_avals.append(jax.core.ShapedArray(shape, dtype))
            zero_outs.append(np.zeros(shape, dtype))
    n_params = len(in_names)
    n_outs = len(out_avals)
    all_in_names = in_names + out_names
    if partition_name is not None:
        all_in_names.append(partition_name)
    donate = tuple(range(n_params, n_params + n_outs))

    def _body(*args):
        operands = list(args)
        if partition_name is not None:
            operands.append(bass2jax.partition_id_tensor())
        outs = bass2jax._bass_exec_p.bind(
            *operands,
            out_avals=tuple(out_avals),
            in_names=tuple(all_in_names),
            out_names=tuple(out_names),
            lowering_input_output_aliases=(),
            sim_require_finite=True,
            sim_require_nnan=True,
            nc=nc,
        )
        return tuple(outs)

    devices = jax.devices()[:n_cores]
    mesh = Mesh(np.asarray(devices), ("core",))
    in_specs = (PartitionSpec("core"),) * (n_params + n_outs)
    out_specs = (PartitionSpec("core"),) * n_outs
    sharded = jax.jit(
        shard_map(
            _body, mesh=mesh, in_specs=in_specs, out_specs=out_specs,
            check_rep=False,
        ),
        donate_argnums=donate,
        keep_unused=True,
    )

    shard = NamedSharding(mesh, PartitionSpec("core"))
    concat_in = [
        jax.device_put(
            np.concatenate([np.asarray(m[nm]) for m in in_maps], axis=0), shard
        )
        for nm in in_names
    ]
    times = []
    out_arrs = None
    for it in range(iters):
        zeros = [
            jax.device_put(
                np.zeros((n_cores * z.shape[0], *z.shape[1:]), z.dtype), shard
            )
            for z in zero_outs
        ]
        jax.block_until_ready(zeros)
        t0 = _time.perf_counter()
        out_arrs = sharded(*concat_in, *zeros)
        jax.block_until_ready(out_arrs)
        times.append(_time.perf_counter() - t0)
    oi = out_names.index("out")
    full = np.asarray(out_arrs[oi]).reshape(n_cores, BPC, HID)
    out = full.reshape(B, HID).reshape(B, 1, HID).astype(np.float32)

    # Device-exec time via chained executions inside one dispatch: the
    # wall-time slope vs chain length isolates on-device time from the
    # axon RPC/dispatch overhead.
    def _chain(n):
        def f(*flat):
            ins = list(flat[:n_params])
            outs = list(flat[n_params:])
            for _ in range(n):
                operands = ins + outs
                if partition_name is not None:
                    operands.append(bass2jax.partition_id_tensor())
                outs = list(
                    bass2jax._bass_exec_p.bind(
                        *operands,
                        out_avals=tuple(out_avals),
                        in_names=tuple(all_in_names),
                        out_names=tuple(out_names),
                        lowering_input_output_aliases=(),
                        sim_require_finite=True,
                        sim_require_nnan=True,
                        nc=nc,
                    )
                )
            return tuple(outs)

        return jax.jit(
            shard_map(
                f, mesh=mesh, in_specs=in_specs, out_specs=out_specs,
                check_rep=False,
            ),
            donate_argnums=donate,
            keep_unused=True,
        )

    chain_times = {}
    for n in (1, 9):
        fn = _chain(n)
        best = None
        for rep in range(4):
            zeros = [
                jax.device_put(
                    np.zeros((n_cores * z.shape[0], *z.shape[1:]), z.dtype),
                    shard,
                )
                for z in zero_outs
            ]
            jax.block_until_ready(zeros)
            t0 = _time.perf_counter()
            r = fn(*concat_in, *zeros)
            jax.block_until_ready(r)
            dt = _time.perf_counter() - t0
            if rep > 0:
                best = dt if best is None else min(best, dt)
        chain_times[n] = best
    exec_ns = (chain_times[9] - chain_times[1]) / 8 * 1e9
    return out, {"iter_walls": times[1:], "chain": chain_times,
                 "exec_ns": exec_ns}



# revision 16
# speedup vs baseline: 1.5386x; 1.5386x over previous
"""Trainium2 Bass kernel for multi-head attention with adaptive span masking.

Computation (per the nn.Module):
    q = (query @ Wq.T) split into B*H rows of size d=64
    attn = softmax((key . q + q @ key_pe) / sqrt(d))
    attn = renormalize(attn * adaptive_span_mask)
    out = (attn . value) merged heads @ Wo.T

Key optimization: the adaptive-span mask is exactly zero for positions
m <= (M-1) - RAMP - span[h]*M, so those key/value rows contribute
nothing to the output (their only coupling is the 1e-8*sum(softmax)
term in the renormalization denominator, which perturbs the result by
~5e-6 relative). Each head therefore only reads the tail [mstart_h, M)
of key/value, cutting HBM traffic by ~2x. mstart_h is computed on the
host from the actual span input and baked into the compiled kernel.

Sharding: batch-parallel across 8 cores. Core c gets batches [4c, 4c+4)
(all 8 heads) = rows [32c, 32c+32) of key/value; Wq/Wo/key_pe/span are
replicated. Each core produces its own [4, 512] output block; the host
concatenates. No collectives needed.

Engine split per (batch, head) row:
  - key AND value loaded via gpsimd (SWDGE) DMA with inline f32->bf16
    cast: HBM reads stay f32 (unavoidable) but no engine time is spent
    casting and SBUF footprint halves
  - QK dot on DVE: bf16 multiply + reduce over d
  - positional scores precomputed per head on PE (key_pe stationary)
  - exp on ACT (with fused sum); mask-mult + sum fused in one DVE
    tensor_tensor_reduce; PV accumulation on PE in bf16
  - prefetch is software-pipelined two heads ahead so the DMA queues
    never drain; masks/iotas are computed in setup so the gpsimd queue
    carries only DMA work in the main loop
"""

import math
import os
import sys

import numpy as np

for _p in ("/opt/trn_rl_repo", "/root/.axon_site/_ro/trn_rl_repo"):
    if os.path.isdir(_p) and _p not in sys.path:
        sys.path.insert(0, _p)

import concourse.bass as bass
import concourse.bacc as bacc
import concourse.mybir as mybir
from concourse.bass import ts
from concourse.masks import make_identity
from concourse.tile import TileContext

F32 = mybir.dt.float32
BF16 = mybir.dt.bfloat16

# Problem constants (hardcoded per contest contract)
NHEADS = 8
HEAD_DIM = 64
HID = NHEADS * HEAD_DIM  # 512
B = 32
M = 8192
RAMP = 32.0

N_CORES = 8
BPC = B // N_CORES        # 4 batches per core
NPC = BPC * NHEADS        # 32 (b,h) rows per core

# tensor_tensor_reduce faults the runtime on this deployment; keep the
# two-op mul+reduce path (opt back in with K_TTR=1 to re-test)
USE_TTR = os.environ.get("K_TTR", "") != ""
USE_CAST_DMA = os.environ.get("K_NO_CAST", "") == ""

_CACHE = {}


def compute_mstarts(span: np.ndarray) -> tuple:
    """First key/value position with nonzero mask, per head, aligned
    down to a multiple of 128 (the SBUF partition count).

    mask[h, m] = clip((m - (M-1) + span[h]*M)/RAMP + 1, 0, 1) is zero
    iff m <= (M-1) - RAMP - span[h]*M.
    """
    s = np.asarray(span, np.float64).reshape(-1)
    last_zero = np.floor((M - 1) - RAMP - s * M).astype(np.int64)
    mstart = np.clip(last_zero, 0, M - 128)
    mstart = (mstart // 128) * 128
    return tuple(int(x) for x in mstart)


def build_nc(mstarts):
    nc = bacc.Bacc(None, target_bir_lowering=False)
    AF = mybir.ActivationFunctionType
    ALU = mybir.AluOpType

    mohs = [(M - mstarts[h]) // 128 for h in range(NHEADS)]
    maxmo = max(mohs)
    # big heads first (their DMA overlaps setup), smallest last (short tail)
    order = sorted(range(NHEADS), key=lambda h: -mohs[h])

    q_d = nc.dram_tensor("query", [BPC, HID], F32, kind="ExternalInput")
    k_d = nc.dram_tensor("key", [NPC, M, HEAD_DIM], F32, kind="ExternalInput")
    v_d = nc.dram_tensor("value", [NPC, M, HEAD_DIM], F32, kind="ExternalInput")
    wq_d = nc.dram_tensor("Wq", [HID, HID], F32, kind="ExternalInput")
    wo_d = nc.dram_tensor("Wo", [HID, HID], F32, kind="ExternalInput")
    kpe_d = nc.dram_tensor("key_pe", [HEAD_DIM, M], F32, kind="ExternalInput")
    span_d = nc.dram_tensor("span", [NHEADS, 1], F32, kind="ExternalInput")
    out_d = nc.dram_tensor("out", [BPC, HID], F32, kind="ExternalOutput")

    with TileContext(nc) as tc:
        with (
            tc.tile_pool(name="persist", bufs=1) as persist,
            # main-loop pools created BEFORE setup pools so the kv DMAs get
            # SBUF ranges disjoint from setup tiles (no WAR dep -> kv loads
            # start at t=0, overlapping the whole setup phase)
            tc.tile_pool(name="kv", bufs=8) as kv_pool,
            tc.tile_pool(name="sc", bufs=3) as sc_pool,
            tc.tile_pool(name="fin", bufs=1) as fin_pool,
            tc.tile_pool(name="ps_pos", bufs=2, space="PSUM") as ps_pos_pool,
            tc.tile_pool(name="ps_s", bufs=2, space="PSUM") as ps_s_pool,
            tc.tile_pool(name="ps_o", bufs=2, space="PSUM") as ps_o_pool,
        ):
            identity = persist.tile([128, 128], F32, tag="identity")
            make_identity(nc, identity[:])
            ones_row = persist.tile([1, 128], F32, tag="ones_row")
            nc.vector.memset(ones_row[:], 1.0)
            ones_col = persist.tile([128, 1], F32, tag="ones_col")
            nc.vector.memset(ones_col[:], 1.0)

            woT = [persist.tile([128, HID], F32, name=f"woT{j}", tag=f"woT{j}") for j in range(4)]
            q_sb = persist.tile([BPC, HID], F32, tag="q_sb")
            qts = persist.tile([HEAD_DIM, NHEADS, BPC], F32, tag="qts")
            qrep = persist.tile([128, BPC, HID], F32, tag="qrep")
            qrep_bf = persist.tile([128, BPC, HID], BF16, tag="qrep_bf")
            # kpe stays f32: a bf16 128-col stationary would trigger the
            # compiler's fast-weight-load path, which requires contiguous
            # weights (ours are strided)
            kpe_sb = persist.tile([HEAD_DIM, M], F32, tag="kpe_sb")
            span_b = persist.tile([128, NHEADS], F32, tag="span_b")
            pos_sb = [
                persist.tile([128, mohs[h], BPC], F32, name=f"pos{h}", tag=f"pos{h}")
                for h in range(NHEADS)
            ]
            masks = [
                persist.tile([128, mohs[h]], F32, name=f"mask{h}", tag=f"mask{h}")
                for h in range(NHEADS)
            ]
            ao_sb = persist.tile([1, BPC, HID], F32, tag="ao_sb")

            # key_pe tail (only columns any head can touch)
            mstart_min = min(mstarts)
            nc.sync.dma_start(
                out=kpe_sb[:, mstart_min:M], in_=kpe_d[:, mstart_min:M]
            )

            # K/V prefetch for one head (4 batch rows), f32->bf16 in-flight
            kv_tiles = {}

            def emit_prefetch(h):
                mo_h = mohs[h]
                mst = mstarts[h]
                tiles = []
                for b in range(BPC):
                    i = b * NHEADS + h
                    if USE_CAST_DMA:
                        kt = kv_pool.tile([128, maxmo, HEAD_DIM], BF16, tag="kt")
                        nc.gpsimd.dma_start(
                            out=kt[:, 0:mo_h, :],
                            in_=k_d[i, mst:M, :].rearrange("(p mo) d -> p mo d", p=128),
                        )
                        vt = kv_pool.tile([128, maxmo, HEAD_DIM], BF16, tag="vt")
                        nc.gpsimd.dma_start(
                            out=vt[:, 0:mo_h, :],
                            in_=v_d[i, mst:M, :].rearrange("(p mo) d -> p mo d", p=128),
                        )
                    else:
                        kt = kv_pool.tile([128, maxmo, HEAD_DIM], F32, tag="kt", bufs=4)
                        nc.sync.dma_start(
                            out=kt[:, 0:mo_h, :],
                            in_=k_d[i, mst:M, :].rearrange("(p mo) d -> p mo d", p=128),
                        )
                        vt = kv_pool.tile([128, maxmo, HEAD_DIM], F32, tag="vt", bufs=4)
                        nc.scalar.dma_start(
                            out=vt[:, 0:mo_h, :],
                            in_=v_d[i, mst:M, :].rearrange("(p mo) d -> p mo d", p=128),
                        )
                    tiles.append((kt, vt))
                kv_tiles[h] = tiles

            emit_prefetch(order[0])
            emit_prefetch(order[1])

            # ---------------- setup phase A: weight transposes + q ----------
            with (
                tc.tile_pool(name="setupA", bufs=1) as sa,
                tc.tile_pool(name="psA", bufs=1, space="PSUM") as psA,
            ):
                wqT = [sa.tile([128, HID], F32, name=f"wqT{j}", tag=f"wqT{j}") for j in range(4)]
                wq_sb = [sa.tile([128, HID], F32, name=f"wq_sb{i}", tag="wq_sb", bufs=2) for i in range(4)]
                wo_sb = [sa.tile([128, HID], F32, name=f"wo_sb{i}", tag="wo_sb", bufs=2) for i in range(4)]
                for i in range(4):
                    nc.sync.dma_start(out=wq_sb[i][:], in_=wq_d[ts(i, 128), :])
                    nc.sync.dma_start(out=wo_sb[i][:], in_=wo_d[ts(i, 128), :])
                for io in range(4):
                    for jo in range(4):
                        pwt = psA.tile([128, 128], F32, tag="pwt")
                        nc.tensor.matmul(
                            pwt[:], wq_sb[io][:, ts(jo, 128)], identity[:],
                            start=True, stop=True,
                        )
                        nc.scalar.copy(wqT[jo][:, ts(io, 128)], pwt[:])
                        pwt2 = psA.tile([128, 128], F32, tag="pwt")
                        nc.tensor.matmul(
                            pwt2[:], wo_sb[io][:, ts(jo, 128)], identity[:],
                            start=True, stop=True,
                        )
                        nc.scalar.copy(woT[jo][:, ts(io, 128)], pwt2[:])

                query_sb = sa.tile([BPC, HID], F32, tag="query_sb")
                nc.sync.dma_start(out=query_sb[:], in_=q_d[:])
                qTq = [sa.tile([128, BPC], F32, name=f"qTq{j}", tag=f"qTq{j}") for j in range(4)]
                for jo in range(4):
                    pqt = psA.tile([128, BPC], F32, tag="pwt")
                    nc.tensor.matmul(
                        pqt[:], query_sb[:, ts(jo, 128)], identity[0:BPC, 0:BPC],
                        start=True, stop=True,
                    )
                    nc.scalar.copy(qTq[jo][:], pqt[:])
                # q = query @ Wq.T  ->  [4, 512]
                ps_q = psA.tile([BPC, HID], F32, tag="ps_q", bufs=1)
                for jo in range(4):
                    nc.tensor.matmul(
                        ps_q[:], qTq[jo][:], wqT[jo][:],
                        start=(jo == 0), stop=(jo == 3),
                    )
                nc.scalar.copy(q_sb[:], ps_q[:])
                # qts[d, h, b] = q[b, h*64+d]   (64 partitions)
                for h in range(NHEADS):
                    pqh = psA.tile([HEAD_DIM, BPC], F32, tag="pwt")
                    nc.tensor.matmul(
                        pqh[:], q_sb[:, ts(h, HEAD_DIM)], identity[0:BPC, 0:BPC],
                        start=True, stop=True,
                    )
                    nc.scalar.copy(qts[:, h, :], pqh[:])

            # ---------------- setup phase B: qrep, span bias, masks ---------
            with (
                tc.tile_pool(name="setupB", bufs=1) as sb,
                tc.tile_pool(name="psB", bufs=1, space="PSUM") as psB,
            ):
                # q replicated across partitions: qrep[p, b, :] = q[b, :]
                # (bounce via DRAM -- DMA partition-broadcast needs a DRAM src)
                with tc.tile_pool(name="dramq", bufs=1, space="DRAM") as dq:
                    q_dram = dq.tile([BPC, HID], F32, tag="q_dram")
                    nc.sync.dma_start(out=q_dram[:], in_=q_sb[:])
                    for b in range(BPC):
                        nc.gpsimd.dma_start(
                            out=qrep[:, b, :],
                            in_=q_dram[b : b + 1, :].partition_broadcast(128),
                        )
                nc.scalar.copy(qrep_bf[:], qrep[:])

                # span bias broadcast to all partitions:
                # span_b[p, h] = span[h]*M/RAMP - (M-1)/RAMP + 1
                span_row = sb.tile([1, NHEADS], F32, tag="span_row")
                nc.sync.dma_start(out=span_row[:], in_=span_d[:].rearrange("h o -> o h"))
                ps_sp = psB.tile([128, NHEADS], F32, tag="ps_sp")
                nc.tensor.matmul(
                    ps_sp[:], ones_row[:], span_row[:], start=True, stop=True
                )
                bias_const = float(-(M - 1) / RAMP + 1.0)
                nc.scalar.activation(
                    out=span_b[:], in_=ps_sp[:], func=AF.Copy,
                    scale=float(M / RAMP), bias=bias_const,
                )

                # adaptive-span masks, one tile per head:
                # masks[h][p, j] = clip((mstart_h + p*mo_h + j)/RAMP
                #                       + span_b[h], 0, 1)
                # the mstart_h/RAMP offset is folded into the per-head bias
                # (iota always starts at 0)
                span_b2 = persist.tile([128, NHEADS], F32, tag="span_b2")
                for h in range(NHEADS):
                    nc.scalar.activation(
                        out=span_b2[:, h : h + 1], in_=span_b[:, h : h + 1],
                        func=AF.Copy, bias=float(mstarts[h] / RAMP),
                    )
                for h in range(NHEADS):
                    mo_h = mohs[h]
                    m_f = sc_pool.tile([128, maxmo], F32, tag="m_f", bufs=2)
                    nc.gpsimd.iota(
                        out=m_f[:, 0:mo_h], pattern=[[1, mo_h]], base=0,
                        channel_multiplier=mo_h,
                        allow_small_or_imprecise_dtypes=True,
                    )
                    nc.scalar.activation(
                        out=masks[h][:], in_=m_f[:, 0:mo_h], func=AF.Identity,
                        scale=float(1.0 / RAMP), bias=span_b2[:, h : h + 1],
                    )
                    nc.vector.tensor_scalar(
                        out=masks[h][:], in0=masks[h][:],
                        scalar1=0.0, scalar2=1.0,
                        op0=ALU.max, op1=ALU.min,
                    )

            # ---------------- main loop: heads, then batches ----------------
            for j, h in enumerate(order):
                mo_h = mohs[h]
                mst = mstarts[h]

                if j + 2 < NHEADS:
                    emit_prefetch(order[j + 2])

                # positional scores for this head:
                # pos[p, mo, b] = sum_d key_pe[d, m] * q[b, h*64+d]
                kpe_r = kpe_sb[:, mst:M].rearrange("d (p mo) -> d mo p", mo=mo_h)
                ps_p = ps_pos_pool.tile([128, maxmo, BPC], F32, tag="ps_p")
                for mo in range(mo_h):
                    nc.tensor.matmul(
                        ps_p[:, mo, :], kpe_r[:, mo, :], qts[:, h, :],
                        start=True, stop=True,
                    )
                nc.scalar.copy(pos_sb[h][:], ps_p[:, 0:mo_h, :])

                for b in range(BPC):
                    kt, vt = kv_tiles[h][b]
                    if not USE_CAST_DMA:
                        vtb = kv_pool.tile([128, maxmo, HEAD_DIM], BF16, tag="vtb", bufs=2)
                        nc.scalar.copy(vtb[:, 0:mo_h, :], vt[:, 0:mo_h, :])
                        vt = vtb
                    # content scores: scores[p, mo] = sum_d key[..] * q[..]
                    prod = sc_pool.tile([128, maxmo, HEAD_DIM], BF16, tag="prod", bufs=2)
                    q_src = qrep_bf if USE_CAST_DMA else qrep
                    q_b = (
                        q_src[:, b, ts(h, HEAD_DIM)]
                        .rearrange("p (x d) -> p x d", x=1)
                        .broadcast_to((128, mo_h, HEAD_DIM))
                    )
                    nc.vector.tensor_mul(prod[:, 0:mo_h, :], kt[:, 0:mo_h, :], q_b)
                    scores = sc_pool.tile([128, maxmo], F32, tag="scores")
                    nc.vector.reduce_sum(
                        scores[:, 0:mo_h], prod[:, 0:mo_h, :], axis=mybir.AxisListType.X
                    )
                    nc.vector.tensor_add(
                        scores[:, 0:mo_h], scores[:, 0:mo_h], pos_sb[h][:, :, b]
                    )
                    # e = exp(scores / sqrt(d)), Sigma_e fused into the ACT op
                    e_t = sc_pool.tile([128, maxmo], F32, tag="e_t")
                    sums = sc_pool.tile([128, 2], F32, tag="sums")
                    nc.scalar.activation(
                        out=e_t[:, 0:mo_h], in_=scores[:, 0:mo_h], func=AF.Exp,
                        scale=float(1.0 / math.sqrt(HEAD_DIM)),
                        accum_out=sums[:, 0:1],
                    )
                    # w = e * mask[h] (bf16 for the PE) + Sigma_w, one DVE op
                    w_t = sc_pool.tile([128, maxmo], BF16, tag="w_t")
                    if USE_TTR:
                        nc.vector.tensor_tensor_reduce(
                            out=w_t[:, 0:mo_h], in0=e_t[:, 0:mo_h], in1=masks[h][:],
                            scale=1.0, scalar=0.0,
                            op0=ALU.mult, op1=ALU.add,
                            accum_out=sums[:, 1:2],
                        )
                    else:
                        nc.vector.tensor_mul(w_t[:, 0:mo_h], e_t[:, 0:mo_h], masks[h][:])
                        nc.vector.reduce_sum(
                            sums[:, 1:2], w_t[:, 0:mo_h], axis=mybir.AxisListType.X
                        )
                    # partition-reduce both sums: [1, 2] = ones.T @ sums
                    ps_s = ps_s_pool.tile([1, 2], F32, tag="ps_s")
                    nc.tensor.matmul(
                        ps_s[:], ones_col[:], sums[:], start=True, stop=True
                    )
                    sums_sb = sc_pool.tile([1, 2], F32, tag="sums_sb")
                    nc.scalar.copy(sums_sb[:], ps_s[:])
                    # u = Sigma_w + 1e-8 * Sigma_e ; scal = 1/u
                    u_t = sc_pool.tile([1, 1], F32, tag="u_t")
                    nc.scalar.activation(
                        out=u_t[:], in_=sums_sb[:, 0:1], func=AF.Identity,
                        scale=1e-8, bias=sums_sb[:, 1:2],
                    )
                    scal = sc_pool.tile([1, 1], F32, tag="scal")
                    nc.vector.reciprocal(scal[:], u_t[:])
                    # out_row = sum_m w[m] * value[m, :]   (bf16 PE, PSUM accum)
                    ps_o = ps_o_pool.tile([1, HEAD_DIM], F32, tag="ps_o")
                    for mo in range(mo_h):
                        nc.tensor.matmul(
                            ps_o[:],
                            w_t[:, mo : mo + 1],
                            vt[:, mo, :],
                            start=(mo == 0),
                            stop=(mo == mo_h - 1),
                        )
                    # ao[0, b, h*64:(h+1)*64] = ps_o * scal
                    nc.scalar.activation(
                        out=ao_sb[0:1, b, ts(h, HEAD_DIM)], in_=ps_o[:],
                        func=AF.Copy, scale=scal[:, 0:1],
                    )

            # ---------------- output projection -------------------------
            with tc.tile_pool(name="ps_fin", bufs=1, space="PSUM") as ps_fin_pool:
                aoT = []
                for co in range(4):
                    ps_t2 = ps_fin_pool.tile([128, BPC], F32, name="ps_t2", tag="ps_fin")
                    for b in range(BPC):
                        nc.tensor.matmul(
                            ps_t2[:, b : b + 1],
                            ao_sb[0:1, b, ts(co, 128)],
                            identity[0:1, 0:1],
                            start=True, stop=True,
                        )
                    t_sb = fin_pool.tile([128, BPC], F32, name=f"t_sb{co}", tag=f"t_sb{co}")
                    nc.scalar.copy(t_sb[:], ps_t2[:])
                    aoT.append(t_sb)
                ps_f = ps_fin_pool.tile([BPC, HID], F32, name="ps_f", tag="ps_fin")
                for co in range(4):
                    nc.tensor.matmul(
                        ps_f[:], aoT[co][:], woT[co][:],
                        start=(co == 0), stop=(co == 3),
                    )
                out_sb = fin_pool.tile([BPC, HID], F32, tag="out_sb")
                nc.scalar.copy(out_sb[:], ps_f[:])
                nc.sync.dma_start(out=out_d[:], in_=out_sb[:])

    nc.compile()
    return nc


def _get_nc(mstarts):
    if mstarts not in _CACHE:
        _CACHE[mstarts] = build_nc(mstarts)
    return _CACHE[mstarts]


def _make_in_maps(query, key, value, Wq, Wo, key_pe, span):
    q2 = np.ascontiguousarray(np.asarray(query, np.float32).reshape(B, HID))
    key = np.asarray(key, np.float32)
    value = np.asarray(value, np.float32)
    Wq = np.ascontiguousarray(np.asarray(Wq, np.float32))
    Wo = np.ascontiguousarray(np.asarray(Wo, np.float32))
    key_pe = np.ascontiguousarray(np.asarray(key_pe, np.float32))
    span = np.ascontiguousarray(np.asarray(span, np.float32))
    in_maps = []
    for c in range(N_CORES):
        in_maps.append(
            {
                "query": np.ascontiguousarray(q2[c * BPC : (c + 1) * BPC]),
                "key": np.ascontiguousarray(key[c * NPC : (c + 1) * NPC]),
                "value": np.ascontiguousarray(value[c * NPC : (c + 1) * NPC]),
                "Wq": Wq,
                "Wo": Wo,
                "key_pe": key_pe,
                "span": span,
            }
        )
    return in_maps


def _install_ntff_hook():
    """Shim antenv.axon_hooks with a ctypes NTFF profile hook so
    run_bass_kernel_spmd(trace=True) works in this container."""
    import contextlib
    import ctypes
    import types

    try:
        import antenv.axon_hooks  # noqa: F401

        return
    except ImportError:
        pass
    so_path = "/opt/axon/libaxon_pjrt.so"
    import antenv

    mod = types.ModuleType("antenv.axon_hooks")
    holder = {"hook": None}

    if os.path.exists(so_path):
        lib = ctypes.CDLL(so_path)
        if hasattr(lib, "axon_start_nrt_profile"):
            lib.axon_start_nrt_profile.argtypes = [
                ctypes.POINTER(ctypes.c_int64),
                ctypes.c_size_t,
            ]
            lib.axon_start_nrt_profile.restype = ctypes.c_int64
            lib.axon_stop_nrt_profile.argtypes = [ctypes.c_char_p]
            lib.axon_stop_nrt_profile.restype = ctypes.c_int64

            @contextlib.contextmanager
            def _hook(output_dir, device_ids):
                import jax

                jax.devices()
                if device_ids:
                    ids = (ctypes.c_int64 * len(device_ids))(*device_ids)
                    rc = lib.axon_start_nrt_profile(ids, len(device_ids))
                else:
                    rc = lib.axon_start_nrt_profile(None, 0)
                if rc != 0:
                    raise RuntimeError(f"axon_start_nrt_profile rc={rc}")
                try:
                    yield
                finally:
                    n = lib.axon_stop_nrt_profile(str(output_dir).encode())
                    print(f"profile: {n} file(s) written to {output_dir}")

            holder["hook"] = _hook

    mod.get_axon_ntff_profile_hook = lambda: holder["hook"]
    mod.set_axon_ntff_profile_hook = lambda h: holder.__setitem__("hook", h)
    sys.modules["antenv.axon_hooks"] = mod
    antenv.axon_hooks = mod


def run(query, key, value, Wq, Wo, key_pe, span, trace=False):
    """Run on hardware; returns (output [B,1,HID], BassKernelResults)."""
    from concourse import bass_utils
    from concourse.bass_utils import run_bass_kernel_spmd

    if trace:
        _install_ntff_hook()
        bass_utils.upload_artifacts = lambda tmpdir: f"local:{tmpdir}"
    nc = _get_nc(compute_mstarts(span))
    in_maps = _make_in_maps(query, key, value, Wq, Wo, key_pe, span)
    res = run_bass_kernel_spmd(nc, in_maps, list(range(N_CORES)), trace=trace)
    out = np.concatenate(
        [np.asarray(res.results[c]["out"]) for c in range(N_CORES)], axis=0
    )
    return out.reshape(B, 1, HID).astype(np.float32), res


def kernel(query, key, value, Wq, Wo, key_pe, span):
    out, _ = run(query, key, value, Wq, Wo, key_pe, span, trace=False)
    return out


# revision 23
# speedup vs baseline: 2.2502x; 1.4625x over previous
"""Trainium2 Bass kernel for multi-head attention with adaptive span masking.

Computation (per the nn.Module):
    q = (query @ Wq.T) split into B*H rows of size d=64
    attn = softmax((key . q + q @ key_pe) / sqrt(d))
    attn = renormalize(attn * adaptive_span_mask)
    out = (attn . value) merged heads @ Wo.T

Key optimization: the adaptive-span mask is exactly zero for positions
m <= (M-1) - RAMP - span[h]*M, so those key/value rows contribute
nothing to the output (their only coupling is the 1e-8*sum(softmax)
term in the renormalization denominator, which perturbs the result by
~5e-6 relative). Each head therefore only reads the tail [mstart_h, M)
of key/value, cutting HBM traffic by ~2x. mstart_h is computed on the
host from the actual span input and baked into the compiled kernel.

Sharding: batch-parallel across 8 cores. Core c gets batches [4c, 4c+4)
(all 8 heads) = rows [32c, 32c+32) of key/value; Wq/Wo/key_pe/span are
replicated. Each core produces its own [4, 512] output block; the host
concatenates. No collectives needed.

Engine split per (batch, head) row:
  - key AND value loaded via gpsimd (SWDGE) DMA with inline f32->bf16
    cast: HBM reads stay f32 (unavoidable) but no engine time is spent
    casting and SBUF footprint halves
  - QK dot on DVE: bf16 multiply + reduce over d
  - positional scores precomputed per head on PE (key_pe stationary)
  - exp on ACT (with fused sum); mask-mult + sum fused in one DVE
    tensor_tensor_reduce; PV accumulation on PE in bf16
  - prefetch is software-pipelined two heads ahead so the DMA queues
    never drain; masks/iotas are computed in setup so the gpsimd queue
    carries only DMA work in the main loop
"""

import math
import os
import sys

import numpy as np

for _p in ("/opt/trn_rl_repo", "/root/.axon_site/_ro/trn_rl_repo"):
    if os.path.isdir(_p) and _p not in sys.path:
        sys.path.insert(0, _p)

import concourse.bass as bass
import concourse.bacc as bacc
import concourse.mybir as mybir
from concourse.bass import ts
from concourse.masks import make_identity
from concourse.tile import TileContext

F32 = mybir.dt.float32
BF16 = mybir.dt.bfloat16

# Problem constants (hardcoded per contest contract)
NHEADS = 8
HEAD_DIM = 64
HID = NHEADS * HEAD_DIM  # 512
B = 32
M = 8192
RAMP = 32.0

N_CORES = 8
BPC = B // N_CORES        # 4 batches per core
NPC = BPC * NHEADS        # 32 (b,h) rows per core

# tensor_tensor_reduce faults the runtime on this deployment; keep the
# two-op mul+reduce path (opt back in with K_TTR=1 to re-test)
USE_TTR = os.environ.get("K_TTR", "") != ""
USE_CAST_DMA = os.environ.get("K_NO_CAST", "") == ""

_CACHE = {}


def compute_mstarts(span: np.ndarray) -> tuple:
    """First key/value position with nonzero mask, per head, aligned
    down to a multiple of 128 (the SBUF partition count).

    mask[h, m] = clip((m - (M-1) + span[h]*M)/RAMP + 1, 0, 1) is zero
    iff m <= (M-1) - RAMP - span[h]*M.
    """
    s = np.asarray(span, np.float64).reshape(-1)
    last_zero = np.floor((M - 1) - RAMP - s * M).astype(np.int64)
    mstart = np.clip(last_zero, 0, M - 128)
    mstart = (mstart // 128) * 128
    return tuple(int(x) for x in mstart)


def build_nc(mstarts):
    nc = bacc.Bacc(None, target_bir_lowering=False)
    AF = mybir.ActivationFunctionType
    ALU = mybir.AluOpType

    mohs = [(M - mstarts[h]) // 128 for h in range(NHEADS)]
    maxmo = max(mohs)
    # big heads first (their DMA overlaps setup), smallest last (short tail)
    order = sorted(range(NHEADS), key=lambda h: -mohs[h])

    q_d = nc.dram_tensor("query", [BPC, HID], F32, kind="ExternalInput")
    k_d = nc.dram_tensor("key", [NPC, M, HEAD_DIM], F32, kind="ExternalInput")
    v_d = nc.dram_tensor("value", [NPC, M, HEAD_DIM], F32, kind="ExternalInput")
    wq_d = nc.dram_tensor("Wq", [HID, HID], F32, kind="ExternalInput")
    wo_d = nc.dram_tensor("Wo", [HID, HID], F32, kind="ExternalInput")
    kpe_d = nc.dram_tensor("key_pe", [HEAD_DIM, M], F32, kind="ExternalInput")
    span_d = nc.dram_tensor("span", [NHEADS, 1], F32, kind="ExternalInput")
    out_d = nc.dram_tensor("out", [BPC, HID], F32, kind="ExternalOutput")

    with TileContext(nc) as tc:
        with (
            tc.tile_pool(name="persist", bufs=1) as persist,
            # main-loop pools created BEFORE setup pools so the kv DMAs get
            # SBUF ranges disjoint from setup tiles (no WAR dep -> kv loads
            # start at t=0, overlapping the whole setup phase)
            tc.tile_pool(name="kv", bufs=8) as kv_pool,
            tc.tile_pool(name="sc", bufs=3) as sc_pool,
            tc.tile_pool(name="fin", bufs=1) as fin_pool,
            tc.tile_pool(name="ps_pos", bufs=2, space="PSUM") as ps_pos_pool,
            tc.tile_pool(name="ps_s", bufs=1, space="PSUM") as ps_s_pool,
            tc.tile_pool(name="ps_o", bufs=2, space="PSUM") as ps_o_pool,
        ):
            identity = persist.tile([128, 128], F32, tag="identity")
            make_identity(nc, identity[:])
            ones_row = persist.tile([1, 128], F32, tag="ones_row")
            nc.vector.memset(ones_row[:], 1.0)
            ones_col = persist.tile([128, 1], F32, tag="ones_col")
            nc.vector.memset(ones_col[:], 1.0)

            woT = [persist.tile([128, HID], F32, name=f"woT{j}", tag=f"woT{j}") for j in range(4)]
            q_sb = persist.tile([BPC, HID], F32, tag="q_sb")
            qts = persist.tile([HEAD_DIM, NHEADS, BPC], F32, tag="qts")
            qts_bf = persist.tile([HEAD_DIM, NHEADS, BPC], BF16, tag="qts_bf")
            qrep_bf = persist.tile([128, BPC, HID], BF16, tag="qrep_bf")
            kpe_sb = persist.tile([HEAD_DIM, M], BF16, tag="kpe_sb")
            span_b = persist.tile([128, NHEADS], F32, tag="span_b")
            span_row = persist.tile([1, NHEADS], F32, tag="span_row")
            pos_sb = [
                persist.tile([128, mohs[h], BPC], F32, name=f"pos{h}", tag=f"pos{h}")
                for h in range(NHEADS)
            ]
            masks = [
                persist.tile([128, mohs[h]], F32, name=f"mask{h}", tag=f"mask{h}")
                for h in range(NHEADS)
            ]
            ao_sb = persist.tile([1, BPC, HID], F32, tag="ao_sb")

            # key_pe tail (only columns any head can touch), cast to bf16
            # in-flight; first instruction on the gpsimd queue
            mstart_min = min(mstarts)
            nc.gpsimd.dma_start(
                out=kpe_sb[:, mstart_min:M], in_=kpe_d[:, mstart_min:M]
            )

            # ---- span bias + adaptive-span masks, computed FIRST so the
            # row pipeline is never gated on them (they feed every row's
            # mask multiply)
            nc.sync.dma_start(out=span_row[:], in_=span_d[:].rearrange("h o -> o h"))
            span_b2 = persist.tile([128, NHEADS], F32, tag="span_b2")
            with tc.tile_pool(name="ps_span", bufs=1, space="PSUM") as ps_span:
                ps_sp = ps_span.tile([128, NHEADS], F32, tag="ps_sp")
                nc.tensor.matmul(
                    ps_sp[:], ones_row[:], span_row[:], start=True, stop=True
                )
                # span_b[p, h] = span[h]*M/RAMP - (M-1)/RAMP + 1;
                # span_b2 folds in the per-head mstart/RAMP offset
                bias_const = float(-(M - 1) / RAMP + 1.0)
                nc.scalar.activation(
                    out=span_b[:], in_=ps_sp[:], func=AF.Copy,
                    scale=float(M / RAMP), bias=bias_const,
                )
            for h in range(NHEADS):
                nc.scalar.activation(
                    out=span_b2[:, h : h + 1], in_=span_b[:, h : h + 1],
                    func=AF.Copy, bias=float(mstarts[h] / RAMP),
                )
            for h in range(NHEADS):
                mo_h = mohs[h]
                m_f = sc_pool.tile([128, maxmo], F32, tag="m_f", bufs=4)
                nc.gpsimd.iota(
                    out=m_f[:, 0:mo_h], pattern=[[1, mo_h]], base=0,
                    channel_multiplier=mo_h,
                    allow_small_or_imprecise_dtypes=True,
                )
                nc.scalar.activation(
                    out=masks[h][:], in_=m_f[:, 0:mo_h], func=AF.Identity,
                    scale=float(1.0 / RAMP), bias=span_b2[:, h : h + 1],
                )
                nc.vector.tensor_scalar(
                    out=masks[h][:], in0=masks[h][:],
                    scalar1=0.0, scalar2=1.0,
                    op0=ALU.max, op1=ALU.min,
                )

            # K/V prefetch for one head (4 batch rows), f32->bf16 in-flight
            kv_tiles = {}

            def emit_prefetch(h):
                mo_h = mohs[h]
                mst = mstarts[h]
                tiles = []
                for b in range(BPC):
                    i = b * NHEADS + h
                    if USE_CAST_DMA:
                        kt = kv_pool.tile([128, maxmo, HEAD_DIM], BF16, tag="kt")
                        nc.gpsimd.dma_start(
                            out=kt[:, 0:mo_h, :],
                            in_=k_d[i, mst:M, :].rearrange("(p mo) d -> p mo d", p=128),
                        )
                        vt = kv_pool.tile([128, maxmo, HEAD_DIM], BF16, tag="vt")
                        nc.gpsimd.dma_start(
                            out=vt[:, 0:mo_h, :],
                            in_=v_d[i, mst:M, :].rearrange("(p mo) d -> p mo d", p=128),
                        )
                    else:
                        kt = kv_pool.tile([128, maxmo, HEAD_DIM], F32, tag="kt", bufs=4)
                        nc.sync.dma_start(
                            out=kt[:, 0:mo_h, :],
                            in_=k_d[i, mst:M, :].rearrange("(p mo) d -> p mo d", p=128),
                        )
                        vt = kv_pool.tile([128, maxmo, HEAD_DIM], F32, tag="vt", bufs=4)
                        nc.scalar.dma_start(
                            out=vt[:, 0:mo_h, :],
                            in_=v_d[i, mst:M, :].rearrange("(p mo) d -> p mo d", p=128),
                        )
                    tiles.append((kt, vt))
                kv_tiles[h] = tiles

            emit_prefetch(order[0])
            emit_prefetch(order[1])

            # ---------------- setup phase A: weight transposes + q ----------
            with (
                tc.tile_pool(name="setupA", bufs=1) as sa,
                tc.tile_pool(name="psA", bufs=2, space="PSUM") as psA,
            ):
                wqT = [sa.tile([128, HID], F32, name=f"wqT{j}", tag=f"wqT{j}") for j in range(4)]
                wq_sb = [sa.tile([128, HID], F32, name=f"wq_sb{i}", tag="wq_sb", bufs=2) for i in range(4)]
                wo_sb = [sa.tile([128, HID], F32, name=f"wo_sb{i}", tag="wo_sb", bufs=2) for i in range(4)]
                for i in range(4):
                    nc.sync.dma_start(out=wq_sb[i][:], in_=wq_d[ts(i, 128), :])
                    nc.sync.dma_start(out=wo_sb[i][:], in_=wo_d[ts(i, 128), :])
                for io in range(4):
                    for jo in range(4):
                        pwt = psA.tile([128, 128], F32, tag="pwt")
                        nc.tensor.matmul(
                            pwt[:], wq_sb[io][:, ts(jo, 128)], identity[:],
                            start=True, stop=True,
                        )
                        nc.scalar.copy(wqT[jo][:, ts(io, 128)], pwt[:])
                        pwt2 = psA.tile([128, 128], F32, tag="pwt")
                        nc.tensor.matmul(
                            pwt2[:], wo_sb[io][:, ts(jo, 128)], identity[:],
                            start=True, stop=True,
                        )
                        nc.scalar.copy(woT[jo][:, ts(io, 128)], pwt2[:])

                query_sb = sa.tile([BPC, HID], F32, tag="query_sb")
                nc.sync.dma_start(out=query_sb[:], in_=q_d[:])
                qTq = [sa.tile([128, BPC], F32, name=f"qTq{j}", tag=f"qTq{j}") for j in range(4)]
                for jo in range(4):
                    pqt = psA.tile([128, BPC], F32, tag="pwt")
                    nc.tensor.matmul(
                        pqt[:], query_sb[:, ts(jo, 128)], identity[0:BPC, 0:BPC],
                        start=True, stop=True,
                    )
                    nc.scalar.copy(qTq[jo][:], pqt[:])
                # q = query @ Wq.T  ->  [4, 512]
                ps_q = psA.tile([BPC, HID], F32, tag="ps_q", bufs=1)
                for jo in range(4):
                    nc.tensor.matmul(
                        ps_q[:], qTq[jo][:], wqT[jo][:],
                        start=(jo == 0), stop=(jo == 3),
                    )
                nc.scalar.copy(q_sb[:], ps_q[:])
                # qts[d, h, b] = q[b, h*64+d]   (64 partitions)
                for h in range(NHEADS):
                    pqh = psA.tile([HEAD_DIM, BPC], F32, tag="pwt")
                    nc.tensor.matmul(
                        pqh[:], q_sb[:, ts(h, HEAD_DIM)], identity[0:BPC, 0:BPC],
                        start=True, stop=True,
                    )
                    nc.scalar.copy(qts[:, h, :], pqh[:])
                nc.scalar.copy(qts_bf[:], qts[:])

            # ---------------- setup phase B: qrep broadcast -----------------
            # qrep_bf[p, b, :] = q[b, :] on every partition: bounce q via
            # DRAM, then ONE sync-queue (HWDGE) broadcast DMA of the flat
            # [1, 2048] row to all 128 partitions + a single ACT cast. The
            # sync queue is free of the K/V flood, so this is ready early.
            qrep = persist.tile([128, BPC * HID], F32, tag="qrep")
            with tc.tile_pool(name="dramq", bufs=1, space="DRAM") as dq:
                q_dram = dq.tile([BPC, HID], F32, tag="q_dram")
                nc.sync.dma_start(out=q_dram[:], in_=q_sb[:])
                nc.sync.dma_start(
                    out=qrep[:],
                    in_=q_dram[:].rearrange("b f -> (b f)").partition_broadcast(128),
                )
            nc.scalar.copy(
                qrep_bf[:], qrep[:].rearrange("p (b f) -> p b f", b=BPC)
            )

            # positional scores for one head, software-pipelined one head
            # ahead of the row compute:
            # pos[p, mo, b] = sum_d key_pe[d, m] * q[b, h*64+d]
            # The stationary kpe slice is strided, so it is loaded as two
            # 64-column halves: a 128-column non-f32 stationary would engage
            # the compiler's fast-weight-load path, which assumes contiguous
            # weights.
            def emit_pos(h):
                mo_h = mohs[h]
                kpe_r = kpe_sb[:, mstarts[h]:M].rearrange(
                    "d (p mo) -> d mo p", mo=mo_h
                )
                ps_p = ps_pos_pool.tile([128, maxmo, BPC], F32, tag="ps_p")
                for mo in range(mo_h):
                    nc.tensor.matmul(
                        ps_p[0:64, mo, :], kpe_r[:, mo, 0:64], qts_bf[:, h, :],
                        start=True, stop=True,
                    )
                    nc.tensor.matmul(
                        ps_p[64:128, mo, :], kpe_r[:, mo, 64:128], qts_bf[:, h, :],
                        start=True, stop=True,
                    )
                nc.scalar.copy(pos_sb[h][:], ps_p[:, 0:mo_h, :])

            emit_pos(order[0])

            # ---------------- main loop: heads, then batches ----------------
            for j, h in enumerate(order):
                mo_h = mohs[h]
                mst = mstarts[h]

                if j + 2 < NHEADS:
                    emit_prefetch(order[j + 2])
                if j + 1 < NHEADS:
                    emit_pos(order[j + 1])

                for b in range(BPC):
                    kt, vt = kv_tiles[h][b]
                    if not USE_CAST_DMA:
                        vtb = kv_pool.tile([128, maxmo, HEAD_DIM], BF16, tag="vtb", bufs=2)
                        nc.scalar.copy(vtb[:, 0:mo_h, :], vt[:, 0:mo_h, :])
                        vt = vtb
                    # content scores: scores[p, mo] = sum_d key[..] * q[..]
                    prod = sc_pool.tile([128, maxmo, HEAD_DIM], BF16, tag="prod", bufs=2)
                    q_b = (
                        qrep_bf[:, b, ts(h, HEAD_DIM)]
                        .rearrange("p (x d) -> p x d", x=1)
                        .broadcast_to((128, mo_h, HEAD_DIM))
                    )
                    nc.vector.tensor_mul(prod[:, 0:mo_h, :], kt[:, 0:mo_h, :], q_b)
                    scores = sc_pool.tile([128, maxmo], F32, tag="scores")
                    nc.vector.reduce_sum(
                        scores[:, 0:mo_h], prod[:, 0:mo_h, :], axis=mybir.AxisListType.X
                    )
                    nc.vector.tensor_add(
                        scores[:, 0:mo_h], scores[:, 0:mo_h], pos_sb[h][:, :, b]
                    )
                    # e = exp(scores / sqrt(d)), Sigma_e fused into the ACT op
                    e_t = sc_pool.tile([128, maxmo], F32, tag="e_t")
                    sums = sc_pool.tile([128, 2], F32, tag="sums")
                    nc.scalar.activation(
                        out=e_t[:, 0:mo_h], in_=scores[:, 0:mo_h], func=AF.Exp,
                        scale=float(1.0 / math.sqrt(HEAD_DIM)),
                        accum_out=sums[:, 0:1],
                    )
                    # w = e * mask[h] (bf16 for the PE) + Sigma_w, one DVE op
                    w_t = sc_pool.tile([128, maxmo], BF16, tag="w_t")
                    if USE_TTR:
                        nc.vector.tensor_tensor_reduce(
                            out=w_t[:, 0:mo_h], in0=e_t[:, 0:mo_h], in1=masks[h][:],
                            scale=1.0, scalar=0.0,
                            op0=ALU.mult, op1=ALU.add,
                            accum_out=sums[:, 1:2],
                        )
                    else:
                        nc.vector.tensor_mul(w_t[:, 0:mo_h], e_t[:, 0:mo_h], masks[h][:])
                        nc.vector.reduce_sum(
                            sums[:, 1:2], w_t[:, 0:mo_h], axis=mybir.AxisListType.X
                        )
                    # partition-reduce both sums: [1, 2] = ones.T @ sums
                    ps_s = ps_s_pool.tile([1, 2], F32, tag="ps_s")
                    nc.tensor.matmul(
                        ps_s[:], ones_col[:], sums[:], start=True, stop=True
                    )
                    sums_sb = sc_pool.tile([1, 2], F32, tag="sums_sb")
                    nc.scalar.copy(sums_sb[:], ps_s[:])
                    # u = Sigma_w + 1e-8 * Sigma_e ; scal = 1/u
                    u_t = sc_pool.tile([1, 1], F32, tag="u_t")
                    nc.scalar.activation(
                        out=u_t[:], in_=sums_sb[:, 0:1], func=AF.Identity,
                        scale=1e-8, bias=sums_sb[:, 1:2],
                    )
                    scal = sc_pool.tile([1, 1], F32, tag="scal")
                    nc.vector.reciprocal(scal[:], u_t[:])
                    # out_row = sum_m w[m] * value[m, :]   (bf16 PE, PSUM accum)
                    ps_o = ps_o_pool.tile([1, HEAD_DIM], F32, tag="ps_o")
                    for mo in range(mo_h):
                        nc.tensor.matmul(
                            ps_o[:],
                            w_t[:, mo : mo + 1],
                            vt[:, mo, :],
                            start=(mo == 0),
                            stop=(mo == mo_h - 1),
                        )
                    # ao[0, b, h*64:(h+1)*64] = ps_o * scal
                    nc.scalar.activation(
                        out=ao_sb[0:1, b, ts(h, HEAD_DIM)], in_=ps_o[:],
                        func=AF.Copy, scale=scal[:, 0:1],
                    )

            # ---------------- output projection -------------------------
            with tc.tile_pool(name="ps_fin", bufs=1, space="PSUM") as ps_fin_pool:
                aoT = []
                for co in range(4):
                    ps_t2 = ps_fin_pool.tile([128, BPC], F32, name="ps_t2", tag="ps_fin")
                    for b in range(BPC):
                        nc.tensor.matmul(
                            ps_t2[:, b : b + 1],
                            ao_sb[0:1, b, ts(co, 128)],
                            identity[0:1, 0:1],
                            start=True, stop=True,
                        )
                    t_sb = fin_pool.tile([128, BPC], F32, name=f"t_sb{co}", tag=f"t_sb{co}")
                    nc.scalar.copy(t_sb[:], ps_t2[:])
                    aoT.append(t_sb)
                ps_f = ps_fin_pool.tile([BPC, HID], F32, name="ps_f", tag="ps_fin")
                for co in range(4):
                    nc.tensor.matmul(
                        ps_f[:], aoT[co][:], woT[co][:],
                        start=(co == 0), stop=(co == 3),
                    )
                out_sb = fin_pool.tile([BPC, HID], F32, tag="out_sb")
                nc.scalar.copy(out_sb[:], ps_f[:])
                nc.sync.dma_start(out=out_d[:], in_=out_sb[:])

    nc.compile()
    return nc


def _get_nc(mstarts):
    if mstarts not in _CACHE:
        _CACHE[mstarts] = build_nc(mstarts)
    return _CACHE[mstarts]


def _make_in_maps(query, key, value, Wq, Wo, key_pe, span):
    q2 = np.ascontiguousarray(np.asarray(query, np.float32).reshape(B, HID))
    key = np.asarray(key, np.float32)
    value = np.asarray(value, np.float32)
    Wq = np.ascontiguousarray(np.asarray(Wq, np.float32))
    Wo = np.ascontiguousarray(np.asarray(Wo, np.float32))
    key_pe = np.ascontiguousarray(np.asarray(key_pe, np.float32))
    span = np.ascontiguousarray(np.asarray(span, np.float32))
    in_maps = []
    for c in range(N_CORES):
        in_maps.append(
            {
                "query": np.ascontiguousarray(q2[c * BPC : (c + 1) * BPC]),
                "key": np.ascontiguousarray(key[c * NPC : (c + 1) * NPC]),
                "value": np.ascontiguousarray(value[c * NPC : (c + 1) * NPC]),
                "Wq": Wq,
                "Wo": Wo,
                "key_pe": key_pe,
                "span": span,
            }
        )
    return in_maps


def _install_ntff_hook():
    """Shim antenv.axon_hooks with a ctypes NTFF profile hook so
    run_bass_kernel_spmd(trace=True) works in this container."""
    import contextlib
    import ctypes
    import types

    try:
        import antenv.axon_hooks  # noqa: F401

        return
    except ImportError:
        pass
    so_path = "/opt/axon/libaxon_pjrt.so"
    import antenv

    mod = types.ModuleType("antenv.axon_hooks")
    holder = {"hook": None}

    if os.path.exists(so_path):
        lib = ctypes.CDLL(so_path)
        if hasattr(lib, "axon_start_nrt_profile"):
            lib.axon_start_nrt_profile.argtypes = [
                ctypes.POINTER(ctypes.c_int64),
                ctypes.c_size_t,
            ]
            lib.axon_start_nrt_profile.restype = ctypes.c_int64
            lib.axon_stop_nrt_profile.argtypes = [ctypes.c_char_p]
            lib.axon_stop_nrt_profile.restype = ctypes.c_int64

            @contextlib.contextmanager
            def _hook(output_dir, device_ids):
                import jax

                jax.devices()
                if device_ids:
                    ids = (ctypes.c_int64 * len(device_ids))(*device_ids)
                    rc = lib.axon_start_nrt_profile(ids, len(device_ids))
                else:
                    rc = lib.axon_start_nrt_profile(None, 0)
                if rc != 0:
                    raise RuntimeError(f"axon_start_nrt_profile rc={rc}")
                try:
                    yield
                finally:
                    n = lib.axon_stop_nrt_profile(str(output_dir).encode())
                    print(f"profile: {n} file(s) written to {output_dir}")

            holder["hook"] = _hook

    mod.get_axon_ntff_profile_hook = lambda: holder["hook"]
    mod.set_axon_ntff_profile_hook = lambda h: holder.__setitem__("hook", h)
    sys.modules["antenv.axon_hooks"] = mod
    antenv.axon_hooks = mod


def run(query, key, value, Wq, Wo, key_pe, span, trace=False):
    """Run on hardware; returns (output [B,1,HID], BassKernelResults)."""
    from concourse import bass_utils
    from concourse.bass_utils import run_bass_kernel_spmd

    if trace:
        _install_ntff_hook()
        bass_utils.upload_artifacts = lambda tmpdir: f"local:{tmpdir}"
    nc = _get_nc(compute_mstarts(span))
    in_maps = _make_in_maps(query, key, value, Wq, Wo, key_pe, span)
    res = run_bass_kernel_spmd(nc, in_maps, list(range(N_CORES)), trace=trace)
    out = np.concatenate(
        [np.asarray(res.results[c]["out"]) for c in range(N_CORES)], axis=0
    )
    return out.reshape(B, 1, HID).astype(np.float32), res


def kernel(query, key, value, Wq, Wo, key_pe, span):
    out, _ = run(query, key, value, Wq, Wo, key_pe, span, trace=False)
    return out
